# revision 31
# baseline (speedup 1.0000x reference)
"""Chamfer distance kernel for 8 Trainium2 NeuronCores (Bass/Tile).

Problem: xyz1, xyz2: (4, 8192, 3) fp32. Outputs dist1, dist2: (4, 8192) fp32,
the row-wise / column-wise minima of the pairwise squared-distance matrix
d[n,m] = max(||x_n||^2 + ||y_m||^2 - 2 x_n.y_m, 0), per batch.

Sharding: core c handles batch c//2 and half of the N rows (c%2). Each core
computes dist1 for its 4096 rows exactly, and a dist2 partial (min over its
4096 rows) for all 8192 columns; the host min-combines the two partials.

Per-core kernel (ONE orientation only — d computed once):
  - distance tiles [128, FD] are produced by ONE bf16 matmul each, using
    K=24 augmented vectors: bf16x3 decompositions of x, of -2*y and of the
    two squared norms, ordered so the large terms cancel early in the fp32
    PSUM accumulation (fp32-faithful, representation residual ~2^-27).
  - HW-measured drain costs (FD=2048 cols of 128 lanes):
      DVE tensor_reduce (PSUM fp32)           ~2069 ns
      DVE tensor_tensor fp16 SBUF (2x mode)    ~927 ns
      ScalarE activation PSUM fp32 -> fp16    ~2287 ns
      DVE tensor_tensor_scan                  ~4500 ns  (NO dual-stream win)
    so the drain is split: ScalarE converts every group to fp16 in SBUF
    (func=Copy; the relu clamp moves to the tiny final tensors, exact since
    min-then-clamp == clamp-then-min), and the DVE does all min work on
    fp16 at 2x:
      dist1 (row-min): elementwise TT-min tree across the tile's groups,
        then halvings down to 64 cols; per-tile tails are folded by one
        final strided tensor_reduce.
      dist2 (col-min): acc_g = min(acc_g, cv_g) fp16 accumulators [128, M]
        (first use is a copy — no init needed), partition-folded by PE
        transposes (identity matmul, fp16 PSUM output via bitcast) +
        strided reduces of the transposed blocks; the fold is emitted
        per-group during the last tile so transposes overlap the tree.
    ScalarE (~2.07us/group) paces the steady state; DVE (~1.86us/group)
    and the PE (~0.85us/group) hide under it. direct_every/dvecvt_every
    (DVE-from-PSUM draining to rebalance engines) measured SLOWER on HW
    (PSUM-ring stalls starve ScalarE) and default off.
"""

from contextlib import ExitStack

import numpy as np
import ml_dtypes

B, N, M = 4, 8192, 8192
NCORES = 8
NLOC = N // 2          # rows of xyz1 per core
P = 128                # partitions
FD = 512               # matmul free dim (one PSUM bank of fp32)
KAUG = 24
BIG = 3.0e38

_BF16 = ml_dtypes.bfloat16


def _decomp3(v):
    """fp32/fp64 array -> three bf16 planes summing to v (residual ~2^-27)."""
    v = v.astype(np.float32)
    h = v.astype(_BF16)
    r = v - h.astype(np.float32)
    m = r.astype(_BF16)
    r2 = r - m.astype(np.float32)
    l = r2.astype(_BF16)
    return h, m, l


def _build_aug(x, y):
    """x: [Nl,3] fp32, y: [Mm,3] fp32 -> (xa [KAUG,Nl] bf16, ya [KAUG,Mm] bf16).

    d[n,m] = sum_k xa[k,n]*ya[k,m] up to bf16x3 residuals. Slot order puts the
    large mutually-cancelling terms first so fp32 PSUM accumulation stays
    accurate near d ~ 0.
    """
    nl, mm = x.shape[0], y.shape[0]
    nx = (x.astype(np.float64) ** 2).sum(axis=1)
    ny = (y.astype(np.float64) ** 2).sum(axis=1)
    xh, xm, xl = _decomp3(x)
    y2 = (-2.0 * y.astype(np.float64)).astype(np.float32)
    yh, ym, yl = _decomp3(y2)
    nxh, nxm, nxl = _decomp3(nx)
    nyh, nym, nyl = _decomp3(ny)

    one_n = np.ones(nl, dtype=_BF16)
    one_m = np.ones(mm, dtype=_BF16)

    xa = np.empty((KAUG, nl), dtype=_BF16)
    ya = np.empty((KAUG, mm), dtype=_BF16)
    k = 0

    def slot(xv, yv):
        nonlocal k
        xa[k] = xv
        ya[k] = yv
        k += 1

    slot(nxh, one_m)
    slot(one_n, nyh)
    for c in range(3):
        slot(xh[:, c], yh[:, c])
    slot(nxm, one_m)
    slot(one_n, nym)
    for c in range(3):
        slot(xh[:, c], ym[:, c])
    for c in range(3):
        slot(xm[:, c], yh[:, c])
    slot(nxl, one_m)
    slot(one_n, nyl)
    for c in range(3):
        slot(xh[:, c], yl[:, c])
    for c in range(3):
        slot(xm[:, c], ym[:, c])
    for c in range(3):
        slot(xl[:, c], yh[:, c])
    assert k == KAUG
    return xa, ya


def build_bass(
    nloc=NLOC, m_total=M, repeat=1, grp=4, psum_bufs=2,
    c_bufs=5, c_pad=16, tail_stop=64, direct_every=0, acc_split=1,
    dvecvt_every=0, t_bufs=2, fold_rate=2,
):
    """Build + compile the per-core Bass program.

    repeat>1 wraps the main compute in a dynamic loop executing it `repeat`
    times — used only to measure per-iteration HW time above the PJRT
    dispatch noise floor.

    direct_every: every direct_every-th (tile, group) is drained directly
    from PSUM by the DVE (reduce + TT col-acc) instead of being converted
    by ScalarE — load-balances the two drain engines. 0 disables.
    """
    import concourse.bacc as bacc
    import concourse.tile as tile
    import concourse.mybir as mybir

    f32 = mybir.dt.float32
    f16 = mybir.dt.float16
    bf16 = mybir.dt.bfloat16
    Alu = mybir.AluOpType
    Act = mybir.ActivationFunctionType
    X = mybir.AxisListType.X

    GFD = grp * FD                   # columns per reduce group
    ntile = nloc // P                # weight tiles (dist1 rows): 32
    ngrp = m_total // GFD            # reduce groups per weight tile: 4
    nblk = m_total // P              # dist2 output blocks: 64
    BIG16 = 6.0e4                    # > any distance, fp16-representable

    nc = bacc.Bacc("TRN2", target_bir_lowering=False, debug=False)
    xa_d = nc.dram_tensor("xa", [KAUG, nloc], bf16, kind="ExternalInput")
    ya_d = nc.dram_tensor("ya", [KAUG, m_total], bf16, kind="ExternalInput")
    id_d = nc.dram_tensor("ident", [P, P], f16, kind="ExternalInput")
    d1_d = nc.dram_tensor("d1", [P, ntile], f32, kind="ExternalOutput")
    d2_d = nc.dram_tensor("d2", [P, nblk], f32, kind="ExternalOutput")

    with tile.TileContext(nc) as tc, ExitStack() as ctx:
        singles = ctx.enter_context(tc.tile_pool(name="singles", bufs=1))
        psum = ctx.enter_context(
            tc.tile_pool(name="psum", bufs=psum_bufs, space="PSUM")
        )

        # chunked loads so the first matmuls start before the full tensors land
        xa = singles.tile([KAUG, nloc], bf16)
        for i in range(4):
            sl = slice(i * nloc // 4, (i + 1) * nloc // 4)
            nc.sync.dma_start(out=xa[:, sl], in_=xa_d.ap()[:, sl])
        ya = singles.tile([KAUG, m_total], bf16)
        for i in range(4):
            sl = slice(i * m_total // 4, (i + 1) * m_total // 4)
            nc.sync.dma_start(out=ya[:, sl], in_=ya_d.ap()[:, sl])
        ident = singles.tile([P, P], f16)
        nc.sync.dma_start(out=ident, in_=id_d.ap())

        cpool = ctx.enter_context(tc.tile_pool(name="cpool", bufs=c_bufs))
        tpool = ctx.enter_context(tc.tile_pool(name="tpool", bufs=t_bufs))

        # dist2 column-min accumulators, one per (tile-range, group): [P, GFD]
        # fp16. acc_split>1 splits the tile range so the first ranges'
        # partition-fold (PE transposes + reduces) overlaps later tiles'
        # compute instead of serializing at the end.
        accs_s = [
            [
                singles.tile([P, GFD], f16, name=f"acc{s}_{g}")
                for g in range(ngrp)
            ]
            for s in range(acc_split)
        ]
        # per-tile row-min tails (tail_stop cols each)
        tails = singles.tile([P, ntile * tail_stop], f16)
        # row-min results of 'direct' groups land here (BIG16 elsewhere)
        dcols_n = max(1, ntile * ngrp)
        dcols = singles.tile([P, dcols_n], f32)

        d1t = singles.tile([P, ntile], f32)
        d2t = singles.tile([P, nblk], f32)
        d2x = singles.tile([P, nblk], f32)

        pending_folds = []

        def enqueue_fold(accs, first):
            """Queue the partition-fold of one acc set as per-pt-slot blocks
            (4 PE transposes + 1 strided reduce each). Blocks are drained one
            per tile boundary so the PE transpose bursts never starve
            ScalarE behind the shared PSUM ring."""
            out = d2t if first else d2x
            blocks = [
                (accs[g], g, t0)
                for g in range(ngrp)
                for t0 in range(0, GFD // P, 4)
            ]

            def mk(block, last):
                acc_g, g, t0 = block

                def emit():
                    pt = psum.tile([P, grp, FD], f32, name="pt", tag="pt")
                    nt = min(4, GFD // P - t0)
                    for j in range(nt):
                        tp = pt[:, j, : P // 2].bitcast(f16)
                        t = t0 + j
                        nc.tensor.transpose(
                            tp, acc_g[:, t * P : (t + 1) * P], ident
                        )
                    blk = g * (GFD // P) + t0
                    nc.vector.tensor_reduce(
                        out=out[:, blk : blk + nt],
                        in_=pt[:, :nt, : P // 2].bitcast(f16),
                        axis=X,
                        op=Alu.min,
                    )
                    if last and not first:
                        nc.vector.tensor_tensor(
                            out=d2t, in0=d2t, in1=d2x, op=Alu.min
                        )

                return emit

            for i, b in enumerate(blocks):
                pending_folds.append(mk(b, i == len(blocks) - 1))

        def enqueue_fold_g(accs, g):
            """Per-group fold enqueue for the single-split case: lets the
            last tile's finished groups start transposing while the DVE is
            still on the tile's row-min tree."""
            for t0 in range(0, GFD // P, 4):
                acc_g = accs[g]

                def emit(acc_g=acc_g, g=g, t0=t0):
                    pt = psum.tile([P, grp, FD], f32, name="pt", tag="pt")
                    nt = min(4, GFD // P - t0)
                    for j in range(nt):
                        tp = pt[:, j, : P // 2].bitcast(f16)
                        t = t0 + j
                        nc.tensor.transpose(
                            tp, acc_g[:, t * P : (t + 1) * P], ident
                        )
                    blk = g * (GFD // P) + t0
                    nc.vector.tensor_reduce(
                        out=d2t[:, blk : blk + nt],
                        in_=pt[:, :nt, : P // 2].bitcast(f16),
                        axis=X,
                        op=Alu.min,
                    )

                pending_folds.append(emit)

        def main_compute():
            # acc tiles need no init: the first col-acc per (split, group) is
            # a straight copy (4x-mode, cheaper than the TT-min it replaces).
            acc_used = [[False] * ngrp for _ in range(acc_split)]
            if direct_every:
                nc.vector.memset(dcols, BIG)
            bounds = [ntile * (s + 1) // acc_split for s in range(acc_split)]

            def col_acc(split, g, src):
                if acc_used[split][g]:
                    nc.vector.tensor_tensor(
                        out=accs_s[split][g], in0=src, in1=accs_s[split][g],
                        op=Alu.min,
                    )
                else:
                    nc.vector.tensor_copy(out=accs_s[split][g], in_=src)
                    acc_used[split][g] = True

            gidx = 0
            for it in range(ntile):
                split = next(s for s, b in enumerate(bounds) if it < b)
                accs = accs_s[split]
                cvs = []
                for g in range(ngrp):
                    pt = psum.tile([P, grp, FD], f32, name="pt", tag="pt")
                    for j in range(grp):
                        nc.tensor.matmul(
                            pt[:, j, :],
                            xa[:, it * P : (it + 1) * P],
                            ya[:, (g * grp + j) * FD : (g * grp + j + 1) * FD],
                            start=True,
                            stop=True,
                        )
                    gidx += 1
                    direct = direct_every and (gidx % direct_every == 0)
                    if dvecvt_every and gidx % dvecvt_every == 0 and not direct:
                        # DVE-side convert: offloads ScalarE (the pacing
                        # engine) using DVE slack; same PSUM slot hold time.
                        cv = cpool.tile([P, GFD + c_pad], f16, name="cv", tag="cv")
                        nc.vector.tensor_copy(
                            out=cv[:, :GFD],
                            in_=pt.rearrange("p g f -> p (g f)"),
                        )
                        col_acc(split, g, cv[:, :GFD])
                        cvs.append(cv)
                    elif direct:
                        ptf = pt.rearrange("p g f -> p (g f)")
                        # row-min straight from PSUM
                        nc.vector.tensor_reduce(
                            out=dcols[:, gidx - 1 : gidx],
                            in_=ptf, axis=X, op=Alu.min,
                        )
                        # col-acc straight from PSUM
                        col_acc(split, g, ptf)
                        cvs.append(None)
                    else:
                        cv = cpool.tile([P, GFD + c_pad], f16, name="cv", tag="cv")
                        # Copy (not Relu): ~130ns/instr cheaper (no bias AP);
                        # the relu clamp moves to the tiny final tensors,
                        # which is exact since min-then-clamp == clamp-then-min.
                        nc.scalar.activation(
                            out=cv[:, :GFD].rearrange("p (g f) -> p g f", g=grp),
                            in_=pt,
                            func=Act.Copy,
                        )
                        # dist2 column accumulate (fp16 TT, 2x mode)
                        col_acc(split, g, cv[:, :GFD])
                        cvs.append(cv)

                    if acc_split == 1 and it == ntile - 1:
                        enqueue_fold_g(accs, g)
                        for _ in range(fold_rate):
                            if pending_folds:
                                pending_folds.pop(0)()

                # dist1 row-min tree over this tile's converted groups
                live = [cv[:, :GFD] for cv in cvs if cv is not None]
                lvl = 0
                while len(live) > 1:
                    nxt = []
                    for i in range(0, len(live) - 1, 2):
                        w = int(live[i].shape[1])
                        o = tpool.tile([P, w], f16, name=f"tL{lvl}", tag=f"tL{lvl}")
                        nc.vector.tensor_tensor(
                            out=o, in0=live[i], in1=live[i + 1], op=Alu.min
                        )
                        nxt.append(o)
                    if len(live) % 2:
                        nxt.append(live[-1])
                    live = nxt
                    lvl += 1
                if live:
                    h = live[0]
                    w = int(h.shape[1])
                    while w > 2 * tail_stop:
                        o = tpool.tile([P, w // 2], f16, name=f"th{w}", tag=f"th{w}")
                        nc.vector.tensor_tensor(
                            out=o, in0=h[:, : w // 2], in1=h[:, w // 2 :],
                            op=Alu.min,
                        )
                        h, w = o, w // 2
                    ts = slice(it * tail_stop, (it + 1) * tail_stop)
                    nc.vector.tensor_tensor(
                        out=tails[:, ts], in0=h[:, :tail_stop],
                        in1=h[:, tail_stop:], op=Alu.min,
                    )
                else:  # all groups direct: nothing converted this tile
                    nc.vector.memset(tails[:, it * tail_stop:(it + 1) * tail_stop], BIG16)

                if it + 1 in bounds and acc_split > 1:
                    enqueue_fold(accs, first=(split == 0))
                if pending_folds:
                    pending_folds.pop(0)()

            # ---- finals ----
            # dist1 finals first: their DVE work fills the gap while the PE
            # runs the remaining fold transposes.
            nc.vector.tensor_reduce(
                out=d1t,
                in_=tails.rearrange("p (t s) -> p t s", s=tail_stop),
                axis=X,
                op=Alu.min,
            )
            if direct_every:
                dmin = singles.tile([P, ntile], f32)
                nc.vector.tensor_reduce(
                    out=dmin,
                    in_=dcols.rearrange("p (t g) -> p t g", g=ngrp),
                    axis=X,
                    op=Alu.min,
                )
                nc.vector.tensor_tensor(out=d1t, in0=d1t, in1=dmin, op=Alu.min)
            nc.vector.tensor_scalar_max(out=d1t, in0=d1t, scalar1=0.0)

            while pending_folds:
                pending_folds.pop(0)()
            nc.vector.tensor_scalar_max(out=d2t, in0=d2t, scalar1=0.0)

        if repeat == 1:
            main_compute()
        else:
            with tc.For_i(0, repeat, 1):
                main_compute()

        nc.sync.dma_start(out=d1_d.ap(), in_=d1t)
        nc.sync.dma_start(out=d2_d.ap(), in_=d2t)

    nc.compile()
    return nc


_CACHED_NC = None


def _get_nc():
    global _CACHED_NC
    if _CACHED_NC is None:
        _CACHED_NC = build_bass()
    return _CACHED_NC


_IDENT = np.eye(P, dtype=np.float16)


def _make_in_maps(xyz1, xyz2):
    xyz1 = np.asarray(xyz1, dtype=np.float32)
    xyz2 = np.asarray(xyz2, dtype=np.float32)
    in_maps = []
    for c in range(NCORES):
        b, h = divmod(c, 2)
        x = xyz1[b, h * NLOC : (h + 1) * NLOC]
        y = xyz2[b]
        xa, ya = _build_aug(x, y)
        in_maps.append({"xa": xa, "ya": ya, "ident": _IDENT})
    return in_maps


def _unshard(results):
    dist1 = np.empty((B, N), np.float32)
    dist2 = np.empty((B, M), np.float32)
    for c in range(NCORES):
        b, h = divmod(c, 2)
        dist1[b, h * NLOC : (h + 1) * NLOC] = np.asarray(results[c]["d1"]).T.ravel()
        d2p = np.asarray(results[c]["d2"]).T.ravel()
        if h == 0:
            dist2[b] = d2p
        else:
            np.minimum(dist2[b], d2p, out=dist2[b])
    return dist1, dist2


def kernel(xyz1, xyz2):
    from concourse.bass_utils import run_bass_kernel_spmd

    nc = _get_nc()
    in_maps = _make_in_maps(xyz1, xyz2)
    res = run_bass_kernel_spmd(nc, in_maps, core_ids=list(range(NCORES)))
    return _unshard(res.results)
